# revision 20
# baseline (speedup 1.0000x reference)
"""4D conv (VALID, stride 1) + ReLU on 8 trn2 NeuronCores via Bass/Tile.

Problem shapes (hardcoded):
  pic_in: [B=2, C=16, D1=8, D2=8, D3=64, D4=64] f32
  weight: [O=32, I=16, 3, 3, 3, 3] f32
  out:    [B=2, O=32, 6, 6, 62, 62] f32

Strategy:
  - Shard output D3 rows across 8 cores (8 rows each, starts S_STARTS;
    neighbouring shards overlap by up to 1 row so every core runs the
    identical program shape).
  - Host pre-builds, per core, a tap-shifted stack of the input so the
    matmul contraction packs input channels with conv taps across SBUF
    partitions. Weights are pre-transposed to lhsT[(tap,c), k1, (k2,o)].
  - Each PSUM tile z[(k2,o), r, n] for one (b, d1o, d2') accumulates
    matmuls over the k1 taps (free-dim AP offsets). M = 96 packs the k2
    taps so the conv's k2 reduction becomes a shifted add across three
    PSUM tiles on the vector engine, followed by ReLU on scalar.
  - design "v1": K = 48 = C x k3-taps; 9 matmuls (k1 x k4) per z tile.
  - design "v3": K = 128 = C x 8 (k3,k4)-taps; the 9th tap (2,2) is a
    K=16 matmul reading the (2,0)-tap partitions (96:112) at a +2
    column offset. 6 matmuls per z tile. The input tile is split in
    overlapping d1 halves so DMA double-buffers against compute.
  - dtype "f32r": fp32 data, PE rounds to ~13-bit mantissa, 4x faster
    streaming than plain fp32 (1 cycle/row at N>=256).
"""

import numpy as np

import concourse.bacc as bacc
import concourse.mybir as mybir
from concourse.tile import TileContext
from concourse.bass_utils import run_bass_kernel_spmd

F32 = mybir.dt.float32

B, C, D1, D2, D3, D4 = 2, 16, 8, 8, 64, 64
O = 32
D1o, D2o, D3o, D4o = 6, 6, 62, 62
S_STARTS = [0, 8, 15, 23, 31, 39, 46, 54]  # per-core output d3 row starts
ROWS = 8  # output d3 rows per core

TAPS14 = [(k1, k4) for k1 in range(3) for k4 in range(3)]
# v3: 8 taps stacked in partitions; (2,2) handled via offset on the (2,0) rows
TAPS8 = [(k3, k4) for k3 in range(3) for k4 in range(3)][:8]
D1_HALVES = [(0, range(0, 3)), (3, range(3, 6))]  # (d1' base, d1o range)


def _post_add(nc, zs, y_sb, d2o):
    # y[d2o] = relu(z[d2o][k2=0] + z[d2o+1][k2=1] + z[d2o+2][k2=2])
    # HW rule: at most one PSUM input per instruction.
    ys = y_sb[:, d2o]
    nc.scalar.activation(ys, zs[d2o][0:32], mybir.ActivationFunctionType.Copy)
    nc.vector.tensor_add(ys, ys, zs[d2o + 1][32:64])
    nc.vector.tensor_add(ys, ys, zs[d2o + 2][64:96])
    nc.scalar.activation(ys, ys, mybir.ActivationFunctionType.Relu)


def build_program(dtype_mode: str = "f32r", reps: int = 1, loop_n: int = 0,
                  design: str = "v3", tap_outer: bool = False):
    nc = bacc.Bacc("TRN2", target_bir_lowering=False, debug=False)
    mmdt = mybir.dt.float32r if dtype_mode == "f32r" else mybir.dt.float32
    y = nc.dram_tensor("y", [B, O, D1o, D2o, ROWS, D4o], F32, kind="ExternalOutput")

    if design == "v1":
        xs = nc.dram_tensor("xs", [B, 48, D1, D2, ROWS, D4], mmdt, kind="ExternalInput")
        wr = nc.dram_tensor("wr", [48, 3, 3, 96], mmdt, kind="ExternalInput")
    elif design == "v3":
        xs = nc.dram_tensor("xs", [B, 128, D1, D2, ROWS, D4], mmdt, kind="ExternalInput")
        wr = nc.dram_tensor("wr", [128, 3, 2, 96], mmdt, kind="ExternalInput")
    else:  # v4
        xs = nc.dram_tensor("xs", [B, 128, D1, D2, ROWS, D4], mmdt, kind="ExternalInput")
        xs2 = nc.dram_tensor(
            "xs2", [B, 3, 48, 2, D2, ROWS, D4], mmdt, kind="ExternalInput"
        )
        wr = nc.dram_tensor("wr", [128, 4, 96], mmdt, kind="ExternalInput")

    with TileContext(nc) as tc:
        with (
            tc.tile_pool(name="w", bufs=1) as wpool,
            tc.tile_pool(name="x", bufs=1 if design == "v1" else 2) as xpool,
            tc.tile_pool(name="x2", bufs=1) as x2pool,
            tc.tile_pool(name="ps", bufs=8, space="PSUM") as pspool,
            tc.tile_pool(name="yb", bufs=2) as ypool,
        ):
            if design == "v1":
                wt = wpool.tile([48, 3, 3, 96], mmdt)
            elif design == "v3":
                wt = wpool.tile([128, 3, 2, 96], mmdt)
            else:
                wt = wpool.tile([128, 4, 96], mmdt)
            nc.sync.dma_start(out=wt[:], in_=wr.ap())

            def body_v1():
                for b in range(B):
                    xt = xpool.tile([48, D1, D2, ROWS, D4], mmdt, tag="xt")
                    nc.sync.dma_start(out=xt[:], in_=xs.ap()[b])
                    for d1o in range(D1o):
                        if tap_outer:
                            # weight-stationary: all 8 PSUM tiles per tap
                            zs = [
                                pspool.tile([96, ROWS, D4o], F32, tag="z", name=f"z{i}")
                                for i in range(D2)
                            ]
                            for j, (k1, k4) in enumerate(TAPS14):
                                for d2p in range(D2):
                                    nc.tensor.matmul(
                                        zs[d2p][:],
                                        lhsT=wt[:, k1, k4, :],
                                        rhs=xt[:, d1o + k1, d2p, :, k4 : k4 + D4o],
                                        start=(j == 0),
                                        stop=(j == len(TAPS14) - 1),
                                    )
                        else:
                            zs = []
                            for d2p in range(D2):
                                z = pspool.tile([96, ROWS, D4o], F32, tag="z")
                                for j, (k1, k4) in enumerate(TAPS14):
                                    nc.tensor.matmul(
                                        z[:],
                                        lhsT=wt[:, k1, k4, :],
                                        rhs=xt[:, d1o + k1, d2p, :, k4 : k4 + D4o],
                                        start=(j == 0),
                                        stop=(j == len(TAPS14) - 1),
                                    )
                                zs.append(z)
                        y_sb = ypool.tile([O, D2o, ROWS, D4o], F32, tag="ysb")
                        for d2o in range(D2o):
                            _post_add(nc, zs, y_sb, d2o)
                        nc.sync.dma_start(out=y.ap()[b, :, d1o], in_=y_sb[:])

            def body_v3():
                for b in range(B):
                    for h0, d1os in D1_HALVES:
                        xt = xpool.tile([128, 5, D2, ROWS, D4], mmdt, tag="xt")
                        nc.sync.dma_start(out=xt[:], in_=xs.ap()[b, :, h0 : h0 + 5])
                        for d1o in d1os:
                            if tap_outer:
                                zs = [
                                    pspool.tile([96, ROWS, D4o], F32, tag="z", name=f"z{i}")
                                    for i in range(D2)
                                ]
                                for k1 in range(3):
                                    d1l = d1o + k1 - h0
                                    for d2p in range(D2):
                                        nc.tensor.matmul(
                                            zs[d2p][:],
                                            lhsT=wt[:, k1, 0, :],
                                            rhs=xt[:, d1l, d2p, :, 0:D4o],
                                            start=(k1 == 0),
                                            stop=False,
                                        )
                                    for d2p in range(D2):
                                        nc.tensor.matmul(
                                            zs[d2p][:],
                                            lhsT=wt[96:112, k1, 1, :],
                                            rhs=xt[96:112, d1l, d2p, :, 2 : 2 + D4o],
                                            start=False,
                                            stop=(k1 == 2),
                                            tile_position=(96, 0),
                                        )
                            else:
                                zs = []
                                for d2p in range(D2):
                                    z = pspool.tile([96, ROWS, D4o], F32, tag="z")
                                    for k1 in range(3):
                                        d1l = d1o + k1 - h0
                                        nc.tensor.matmul(
                                            z[:],
                                            lhsT=wt[:, k1, 0, :],
                                            rhs=xt[:, d1l, d2p, :, 0:D4o],
                                            start=(k1 == 0),
                                            stop=False,
                                        )
                                        nc.tensor.matmul(
                                            z[:],
                                            lhsT=wt[96:112, k1, 1, :],
                                            rhs=xt[96:112, d1l, d2p, :, 2 : 2 + D4o],
                                            start=False,
                                            stop=(k1 == 2),
                                            tile_position=(96, 0),
                                        )
                                    zs.append(z)
                            y_sb = ypool.tile([O, D2o, ROWS, D4o], F32, tag="ysb")
                            for d2o in range(D2o):
                                _post_add(nc, zs, y_sb, d2o)
                            nc.sync.dma_start(out=y.ap()[b, :, d1o], in_=y_sb[:])

            def body_v4():
                # 4 matmuls per z tile: 3x K=128 (8 taps) + 1x K=48 merging
                # tap (2,2) across the k1 taps via a separate (k1,c)-stacked
                # input. d1 in three 4-row chunks so SBUF fits 2 xt bufs.
                for b in range(B):
                    for ci in range(3):
                        h0 = 2 * ci
                        xt = xpool.tile([128, 4, D2, ROWS, D4], mmdt, tag="xt")
                        nc.sync.dma_start(out=xt[:], in_=xs.ap()[b, :, h0 : h0 + 4])
                        xt2 = x2pool.tile([48, 2, D2, ROWS, D4], mmdt, tag="xt2")
                        nc.sync.dma_start(out=xt2[:], in_=xs2.ap()[b, ci])
                        for d1o in (h0, h0 + 1):
                            dl = d1o - h0
                            zs = [
                                pspool.tile([96, ROWS, D4o], F32, tag="z", name=f"z{i}")
                                for i in range(D2)
                            ]
                            for k1 in range(3):
                                for d2p in range(D2):
                                    nc.tensor.matmul(
                                        zs[d2p][:],
                                        lhsT=wt[:, k1, :],
                                        rhs=xt[:, dl + k1, d2p, :, 0:D4o],
                                        start=(k1 == 0),
                                        stop=False,
                                    )
                            for d2p in range(D2):
                                nc.tensor.matmul(
                                    zs[d2p][:],
                                    lhsT=wt[0:48, 3, :],
                                    rhs=xt2[:, dl, d2p, :, 0:D4o],
                                    start=False,
                                    stop=True,
                                )
                            y_sb = ypool.tile([O, D2o, ROWS, D4o], F32, tag="ysb")
                            for d2o in range(D2o):
                                _post_add(nc, zs, y_sb, d2o)
                            nc.sync.dma_start(out=y.ap()[b, :, d1o], in_=y_sb[:])

            body = {"v1": body_v1, "v3": body_v3, "v4": body_v4}[design]
            if loop_n > 0:
                with tc.For_i(0, loop_n, 1):
                    body()
            else:
                for _rep in range(reps):
                    body()
    nc.compile()
    return nc


def make_in_maps(pic_in: np.ndarray, weight: np.ndarray, design: str = "v3"):
    pic_in = np.ascontiguousarray(pic_in, dtype=np.float32)
    weight = np.asarray(weight, dtype=np.float32)
    in_maps = []
    if design == "v1":
        # lhsT[(k3,c), k1, k4, (k2,o)] = w[o, c, k1, k2, k3, k4]
        wre = np.ascontiguousarray(
            weight.transpose(4, 1, 2, 5, 3, 0).reshape(48, 3, 3, 96)
        )
        for s in S_STARTS:
            xst = np.empty((B, 48, D1, D2, ROWS, D4), np.float32)
            for k3 in range(3):
                xst[:, k3 * 16 : (k3 + 1) * 16] = pic_in[
                    :, :, :, :, s + k3 : s + k3 + ROWS, :
                ]
            in_maps.append({"xs": xst, "wr": wre})
        return in_maps

    # w[o, c, k1, k2, k3, k4] -> wt_k[c, k1, (k2,o), k3, k4]
    wt_k = weight.transpose(1, 2, 3, 0, 4, 5).reshape(16, 3, 96, 3, 3)

    if design == "v3":
        # slot 0 = 8 stacked taps, slot 1 = tap (2,2) on partitions 96:112
        wre = np.zeros((128, 3, 2, 96), np.float32)
        for t, (k3, k4) in enumerate(TAPS8):
            wre[t * 16 : (t + 1) * 16, :, 0, :] = wt_k[:, :, :, k3, k4]
        wre[96:112, :, 1, :] = wt_k[:, :, :, 2, 2]
    else:  # v4: slots 0..2 = per-k1 8-tap weights, slot 3 = (k1,c)-stacked (2,2)
        wre = np.zeros((128, 4, 96), np.float32)
        for t, (k3, k4) in enumerate(TAPS8):
            wre[t * 16 : (t + 1) * 16, 0:3, :] = wt_k[:, :, :, k3, k4].transpose(
                0, 1, 2
            )
        for k1 in range(3):
            wre[k1 * 16 : (k1 + 1) * 16, 3, :] = wt_k[:, k1, :, 2, 2]

    for s in S_STARTS:
        xst = np.zeros((B, 128, D1, D2, ROWS, D4), np.float32)
        for t, (k3, k4) in enumerate(TAPS8):
            xst[:, t * 16 : (t + 1) * 16, :, :, :, : D4 - k4] = pic_in[
                :, :, :, :, s + k3 : s + k3 + ROWS, k4:
            ]
        im = {"xs": xst, "wr": wre}
        if design == "v4":
            xs2 = np.zeros((B, 3, 48, 2, D2, ROWS, D4), np.float32)
            for ci in range(3):
                for k1 in range(3):
                    for dl in range(2):
                        xs2[:, ci, k1 * 16 : (k1 + 1) * 16, dl, :, :, : D4 - 2] = (
                            pic_in[:, :, 2 * ci + dl + k1, :, s + 2 : s + 2 + ROWS, 2:]
                        )
            im["xs2"] = xs2
        in_maps.append(im)
    return in_maps


def assemble_output(results):
    out = np.empty((B, O, D1o, D2o, D3o, D4o), np.float32)
    for i, s in enumerate(S_STARTS):
        out[:, :, :, :, s : s + ROWS, :] = results[i]["y"]
    return out


def kernel(pic_in: np.ndarray, weight: np.ndarray) -> np.ndarray:
    nc = build_program(dtype_mode="f32r", design="v4")
    in_maps = make_in_maps(pic_in, weight, design="v4")
    res = run_bass_kernel_spmd(nc, in_maps, list(range(8)))
    return assemble_output(res.results)
